# revision 12
# baseline (speedup 1.0000x reference)
"""Lennard-Jones pair energies + per-atom segment sum on 8 Trainium2 cores.

Strategy (edge-partitioned, like GNN edge partitioning per the sharding hint):

Host (sharding step): pairs are partitioned by destination atom and packed
into a dense ELL-style layout -- atoms are grouped into sections by their
(padded) pair count L, each section laid out as [blocks, 128, L] so that one
SBUF partition holds one atom's L-slot pair run.  Pad slots use dist=RC, for
which the shifted LJ energy is exactly 0, so pads are numerically inert.
Atom sections are split evenly across the 8 cores (identical section shapes
per core, so one SPMD program serves all cores).

Device (all the arithmetic): for every pair slot computes the shifted LJ
energy and reduces each atom's L-run to its per-atom half-energy:
    en/2 = (v - b)^2 - 1/2 - e0/2,   v = sqrt(2)/d^6,  b = sqrt(1/2)
streamed as: w=d^2 (ACT), x=w^2 (ACT), y=(w*sqrt(.5))*x (DVE stt),
v=1/y (DVE reciprocal), bp=(v-b)^2 (ACT), grouped-sum over L (DVE reduce),
then a per-section constant fixup -L*(1/2+e0/2).

Host (unshard step): scatters the per-atom results back to atom order.
"""

import math

import numpy as np

RC = 3.0
N_CORES = 8
P = 128  # SBUF partitions
PAD_MULT = 4  # per-atom slot-count quantum
F_TARGET = 2048  # target free-dim elements per SBUF tile


def _build_layout(idx: np.ndarray, n_atoms: int):
    """Partition pairs by atom into dense per-core ELL sections.

    Returns (sections, slotmap, atom_of):
      sections: list of (L, m) with m blocks per core, identical across cores
      slotmap:  [N_CORES, W] pair id per slot (-1 = pad)
      atom_of:  [N_CORES, M_out, P] atom id per output cell (-1 = pad)
    """
    counts = np.bincount(idx, minlength=n_atoms)
    perm = np.argsort(idx, kind="stable")
    starts = np.zeros(n_atoms + 1, np.int64)
    starts[1:] = np.cumsum(counts)
    q = ((counts + PAD_MULT - 1) // PAD_MULT) * PAD_MULT
    has = counts > 0
    sections = []
    slot_chunks = [[] for _ in range(N_CORES)]
    atom_chunks = [[] for _ in range(N_CORES)]
    for L in np.unique(q[has]):
        atoms_L = np.where(has & (q == L))[0]
        n = len(atoms_L)
        n_pad = ((n + N_CORES * P - 1) // (N_CORES * P)) * (N_CORES * P)
        m = n_pad // (N_CORES * P)
        sections.append((int(L), int(m)))
        atoms_pad = np.full(n_pad, -1, np.int64)
        atoms_pad[:n] = atoms_L
        offs = np.arange(L)[None, :]
        cnt = np.where(atoms_pad >= 0, counts[np.maximum(atoms_pad, 0)], 0)
        valid = offs < cnt[:, None]
        src = starts[np.maximum(atoms_pad, 0)][:, None] + offs
        pairmat = np.full((n_pad, L), -1, np.int64)
        pairmat[valid] = perm[src[valid]]
        per_core = n_pad // N_CORES
        for c in range(N_CORES):
            chunk = pairmat[c * per_core : (c + 1) * per_core]
            slot_chunks[c].append(chunk.reshape(-1))
            atom_chunks[c].append(
                atoms_pad[c * per_core : (c + 1) * per_core].reshape(m, P)
            )
    slotmap = np.stack([np.concatenate(ch) for ch in slot_chunks])
    atom_of = np.stack([np.concatenate(ch, axis=0) for ch in atom_chunks])
    return sections, slotmap, atom_of


def _build_bass_program(sections, W: int):
    import concourse.bass as bass
    import concourse.tile as tile
    from concourse import bacc, mybir

    f32 = mybir.dt.float32
    e0 = 4.0 * ((1.0 / RC) ** 12 - (1.0 / RC) ** 6)
    beta = math.sqrt(0.5)
    m_total = sum(m for _, m in sections)

    nc = bacc.Bacc(
        "TRN2",
        target_bir_lowering=False,
        debug=False,
        enable_asserts=False,
        num_devices=N_CORES,
    )
    din = nc.dram_tensor("dist_packed", [W], f32, kind="ExternalInput")
    dout = nc.dram_tensor("en_blocks", [P, m_total], f32, kind="ExternalOutput")

    with tile.TileContext(nc) as tc:
        with (
            tc.tile_pool(name="io", bufs=6) as io_pool,
            tc.tile_pool(name="tmp", bufs=3) as tmp_pool,
            tc.tile_pool(name="acc", bufs=1) as acc_pool,
        ):
            out_raw = acc_pool.tile([P, m_total], f32, tag="out_raw")
            out_fin = acc_pool.tile([P, m_total], f32, tag="out_fin")
            nbias = acc_pool.tile([P, 1], f32, tag="nbias")
            nc.vector.memset(nbias[:], -beta)
            mscale = acc_pool.tile([P, 1], f32, tag="mscale")
            nc.vector.memset(mscale[:], -6.0)
            lbias = acc_pool.tile([P, 1], f32, tag="lbias")
            nc.vector.memset(lbias[:], math.log(math.sqrt(2.0)))
            off = 0
            col = 0
            for L, m in sections:
                sec = din.ap()[off : off + m * P * L].rearrange(
                    "(b p l) -> p b l", p=P, l=L
                )
                g_max = max(1, F_TARGET // L)
                b0 = 0
                while b0 < m:
                    g = min(g_max, m - b0)
                    F = g * L
                    d = io_pool.tile([P, F], f32, tag="d_in")
                    nc.sync.dma_start(
                        d[:].rearrange("p (b l) -> p b l", l=L),
                        sec[:, b0 : b0 + g, :],
                    )
                    # v = sqrt(2)/d^6 = exp(-6*ln(d) + ln(sqrt(2)))
                    t = tmp_pool.tile([P, F], f32, tag="t")
                    nc.scalar.activation(
                        t[:], d[:], mybir.ActivationFunctionType.Ln
                    )
                    v = tmp_pool.tile([P, F], f32, tag="v")
                    nc.scalar.activation(
                        v[:],
                        t[:],
                        mybir.ActivationFunctionType.Exp,
                        bias=lbias[:],
                        scale=mscale[:],
                    )
                    bp = tmp_pool.tile([P, F], f32, tag="bp")
                    nc.scalar.activation(
                        bp[:],
                        v[:],
                        mybir.ActivationFunctionType.Square,
                        bias=nbias[:],
                        scale=1.0,
                    )
                    nc.vector.tensor_reduce(
                        out_raw[:, col : col + g],
                        bp[:].rearrange("p (b l) -> p b l", l=L),
                        axis=mybir.AxisListType.X,
                        op=mybir.AluOpType.add,
                    )
                    b0 += g
                    col += g
                # per-section constant fixup: en/2 = sum(bp) - L*(1/2 + e0/2)
                nc.vector.tensor_scalar(
                    out_fin[:, col - m : col],
                    out_raw[:, col - m : col],
                    float(-L * (0.5 + e0 / 2.0)),
                    None,
                    mybir.AluOpType.add,
                )
                off += m * P * L
            nc.sync.dma_start(dout.ap(), out_fin[:])
    nc.compile()
    return nc


def kernel(**inputs) -> np.ndarray:
    dist = np.ascontiguousarray(np.asarray(inputs["dist"], dtype=np.float32))
    ind_2 = np.asarray(inputs["ind_2"])
    n_atoms = int(np.asarray(inputs["ind_1"]).shape[0])
    idx = ind_2[:, 0].astype(np.int64)

    sections, slotmap, atom_of = _build_layout(idx, n_atoms)
    W = slotmap.shape[1]

    in_maps = []
    for c in range(N_CORES):
        sm = slotmap[c]
        packed = np.where(sm >= 0, dist[np.maximum(sm, 0)], np.float32(RC))
        in_maps.append({"dist_packed": np.ascontiguousarray(packed, np.float32)})

    nc = _build_bass_program(sections, W)

    from concourse import bass_utils

    res = bass_utils.run_bass_kernel_spmd(
        nc, in_maps, core_ids=list(range(N_CORES))
    )

    out_full = np.zeros(n_atoms, np.float32)
    for c in range(N_CORES):
        dev = res.results[c]["en_blocks"]  # [P, M_out]
        a = atom_of[c]  # [M_out, P]
        valid = a >= 0
        out_full[a[valid]] = dev.T[valid]
    return out_full
